# revision 1
# baseline (speedup 1.0000x reference)
"""Trainium2 Bass kernel for ModalitySpecificLocalSelfAttention (7x7 local window).

Strategy (8 NeuronCores, spatial-parallel over H):
  - Each core owns a 16-row stripe of the 128x128 image; k/v paths get a
    3-row halo (22 rows total). 1x1 convs run as PE matmuls with BN scale
    folded into the weights on the host (float32r, full PE rate).
  - Local attention per 8x16 pixel block: one matmul Q_blockT @ K_neigh
    gives a dense [128, 308] score matrix covering the 14x22 padded
    neighborhood; a constant additive mask (-1e30 off-window) + exp
    reproduces the reference's zero-pad softmax semantics exactly.
  - Value aggregation: PE-transpose A and V_neigh to put the neighborhood
    dim on partitions (bf16), then one accumulating matmul -> [C, 128].
  - Final conv: W_a @ attn + W_x @ x accumulated in PSUM + bias.
"""

import sys

for _p in ("/opt/trn_rl_repo", "/root/.axon_site/_ro/trn_rl_repo"):
    if _p not in sys.path:
        sys.path.append(_p)

import ml_dtypes
import numpy as np

import concourse.bass as bass
from concourse import mybir
from concourse.bass_utils import run_bass_kernel_spmd

F32 = mybir.dt.float32
F32R = mybir.dt.float32r
BF16 = mybir.dt.bfloat16

C = 128
H = 128
W = 128
NCORES = 8
RPC = H // NCORES          # 16 rows per core
PAD = 3
HALO = RPC + 2 * PAD       # 22 rows incl halo
WP = W + 2 * PAD           # 134 padded width
BR, BC = 8, 16             # pixel block 8 rows x 16 cols = 128 pixels
NBR, NBC = RPC // BR, W // BC
NR, NC_ = BR + 2 * PAD, BC + 2 * PAD  # neighborhood 14 x 22
NN = NR * NC_              # 308
NPIX = RPC * W             # 2048 pixels per core
NHALO = HALO * W           # 2816

EXP_SHIFT = -16.0          # constant bias inside exp (sim range ~[0, 32])
CH = 512                   # conv matmul N-chunk (one PSUM bank of f32)


NN2 = 384                  # attention width padded to XBAR/transpose granularity


def _build_program():
    """Raw-Bass SPMD program (one NeuronCore's 16-row stripe).

    Single-wait discipline: this walrus build rejects instructions with
    more than one semaphore wait, so every cross-engine dependency is a
    standalone ``wait_ge`` and each tracked instruction increments its
    engine's semaphore.  The schedule is planned in Python, then emitted
    into the per-engine streams of an ``nc.Block``.

    Block phase runs in 4 groups of 4 blocks (one 8-row block-row half)
    so softmax ops amortize instruction overhead; A/V transposes run on
    the DMA XBAR (bf16), not the PE.
    """
    nc = bass.Bass("TRN2", target_bir_lowering=False, debug=False)

    # ---- DRAM I/O ----
    xs_d = nc.dram_tensor("xs", [C, NHALO], BF16, kind="ExternalInput").ap()
    W_NAMES = ("wq1t", "wq2t", "wk1t", "wk2t", "wvt", "wat", "wxt")
    B_NAMES = ("bq1", "bq2", "bk1", "bk2", "bv", "bo")
    wall_d = nc.dram_tensor("wall", [C, 8 * C], BF16, kind="ExternalInput").ap()
    ball_d = nc.dram_tensor("ball", [C, 8], F32, kind="ExternalInput").ap()
    smask_d = nc.dram_tensor("smask", [C, NBR, NN2], BF16,
                             kind="ExternalInput").ap()
    y_d = nc.dram_tensor("y", [C, NPIX], F32, kind="ExternalOutput").ap()

    # ---- SBUF ----
    sb = lambda name, shape, dt: nc.alloc_sbuf_tensor(name, list(shape), dt).ap()
    xs = sb("xs_sb", [C, NHALO], BF16)
    k1 = sb("k1_sb", [C, NHALO], BF16)
    q1 = sb("q1_sb", [C, NPIX], BF16)
    q = sb("q_sb", [C, RPC, W], BF16)
    kpad = sb("kpad_sb", [C, HALO, WP], BF16)
    vpad = sb("vpad_sb", [C, HALO, WP], BF16)
    attn = sb("attn_sb", [C, RPC, W], BF16)
    wall = sb("wall_sb", [C, 8 * C], BF16)
    ball = sb("ball_sb", [C, 8], F32)
    w_sb = {n: wall[:, bass.ts(k, C)] for k, n in enumerate(W_NAMES)}
    b_sb = {n: ball[:, k:k + 1] for k, n in enumerate(B_NAMES)}
    id16 = wall[:, bass.ts(7, C)]
    oobc = ball[:, 6:8]
    smask = sb("smask_sb", [C, NBR, NN2], BF16)
    eshift = sb("eshift_sb", [C, 1], F32)
    qb4 = sb("qb4_sb", [C, 2, 4, BR * BC], BF16)
    kn4 = sb("kn4_sb", [C, 2, 4, NN2], BF16)
    vn4 = sb("vn4_sb", [C, 2, 4, NN2], BF16)
    am4 = sb("am4_sb", [C, 2, 4, NN2], BF16)
    e4 = sb("e4_sb", [C, 2, 4, NN2], BF16)
    a16 = sb("a16_sb", [C, 2, 4, NN2], BF16)
    z4 = sb("z4_sb", [C, 2, 4], F32)
    zs4 = sb("zs4_sb", [C, 2, 4], F32)
    rz4 = sb("rz4_sb", [C, 2, 4], F32)
    at16 = sb("at16_sb", [C, 2, 4, 3, C], BF16)
    vt16 = sb("vt16_sb", [C, 2, 4, 3, C], BF16)
    yt = sb("yt_sb", [C, 2, CH], F32)

    # ---- PSUM: two 4-bank tensors = all 8 banks ----
    # conv phase: 8 rotating [C, 512] chunk slots
    # block group g: parity tensor holds 4x S ([.., :384]) + 4x AV ([.., 384:])
    # o-conv: alternating [C, 512] slots
    PS = [nc.alloc_psum_tensor(f"ps{i}", [C, 4, CH], F32).ap()
          for i in range(2)]

    # ---- semaphores ----
    sem_names = tuple(
        ["sde", "sdw", "sdt", "sdout0", "sdout1", "sp", "sa", "sv", "sg"]
        + [f"sdx{j}" for j in range(6)]
        + [f"sdta{p}{i}" for p in range(2) for i in range(4)]
        + [f"sdtv{p}{i}" for p in range(2) for i in range(4)])
    sems = {n: nc.alloc_semaphore(n) for n in sem_names}

    ENGS = ("sync", "pe", "act", "dve", "gp")
    plan = {e: [] for e in ENGS}
    cnt = {n: 0 for n in sem_names}

    def op(eng, fn, sem, inc=1):
        plan[eng].append(("op", fn, sem, inc))
        if sem:
            cnt[sem] += inc
            return cnt[sem]
        return None

    def wait(eng, sem, val):
        if val and val > 0:
            plan[eng].append(("w", sem, val))

    RELU = mybir.ActivationFunctionType.Relu
    IDENT = mybir.ActivationFunctionType.Identity
    EXP = mybir.ActivationFunctionType.Exp

    # ---- input DMAs: early group feeds k1/q1 start, rest follows ----
    def dma_in(sem, dst, srcd):
        return op("sync",
                  lambda d=dst, s=srcd: nc.sync.dma_start(out=d, in_=s),
                  sem, 16)

    dma_in("sde", wall, wall_d)
    SDE_ALL = dma_in("sde", ball, ball_d)
    for i in range((NHALO + CH - 1) // CH):
        n = min(CH, NHALO - i * CH)
        dma_in(f"sdx{i}", xs[:, bass.ds(i * CH, n)],
               xs_d[:, bass.ds(i * CH, n)])
    SDW_ALL = dma_in("sdw", smask, smask_d)

    # ---- init memsets (zero borders of kpad/vpad; zero pad cols of kn/vn) ----
    for t in (kpad, vpad):
        op("gp", lambda tf=t.rearrange("p r w -> p (r w)"):
            nc.gpsimd.memset(tf, 0.0), "sg")
    for t in (kn4, vn4):
        op("gp", lambda tf=t.rearrange("p a b c -> p (a b c)"):
            nc.gpsimd.memset(tf, 0.0), "sg")
    MEMSETS = cnt["sg"]
    op("dve", lambda: nc.vector.memset(eshift, EXP_SHIFT), "sv")

    # ---- convs: emitted chunk-by-chunk in a custom order ----
    conv_list = [
        ("k1", "wk1t", xs, 0, NHALO, k1, "bk1"),
        ("q1", "wq1t", xs, PAD * W, NPIX, q1, "bq1"),
        ("v", "wvt", xs, 0, NHALO, vpad, "bv"),
        ("k2", "wk2t", k1, 0, NHALO, kpad, "bk2"),
        ("q2", "wq2t", q1, 0, NPIX, q, "bq2"),
    ]
    mm_done, epi_done = {}, {}
    last_slot_epi = {}           # (tensor_idx, slot) -> epi mark
    last_tensor_epi = [None, None]   # tensor_idx -> (sem, value)

    def emit_conv_chunk(ci, j, tidx, slot):
        cname, wn, rhs, roff, ntot, dst, bn = conv_list[ci]
        n = min(CH, ntot - j * CH)
        ps = PS[tidx][:, slot, :]
        src_conv = {"k2": 0, "q2": 1}.get(cname)
        if cname in ("k1", "v"):
            wait("pe", f"sdx{j}", 16)
        elif cname == "q1":
            wait("pe", f"sdx{j}", 16)
            wait("pe", f"sdx{j + 1}", 16)
        if src_conv is not None:
            m_ = epi_done.get((src_conv, j))
            if m_:
                wait("pe", m_[0], m_[1])
        m_ = last_slot_epi.get((tidx, slot))
        if m_:
            wait("pe", m_[0], m_[1])
        mm_done[(ci, j)] = op(
            "pe",
            lambda p=ps[:, :n], w_=w_sb[wn],
                   r=rhs[:, bass.ds(roff + j * CH, n)]:
                nc.tensor.matmul(p, w_, r, start=True, stop=True),
            "sp")
        wait("act", "sp", mm_done[(ci, j)])
        if cname in ("v", "k2"):
            nr = n // W
            r0 = (j * CH) // W
            if cname == "v":
                wait("act", "sg", MEMSETS)
            val = ("sa", op(
                "act",
                lambda o=dst[:, r0:r0 + nr, PAD:PAD + W],
                       p=ps[:, :n].rearrange("p (r w) -> p r w", w=W),
                       b=b_sb[bn]:
                    nc.scalar.activation(o, p, RELU, bias=b),
                "sa"))
        elif cname == "q2":
            nr = n // W
            r0 = (j * CH) // W
            val = ("sa", op(
                "act",
                lambda o=dst[:, r0:r0 + nr, :],
                       p=ps[:, :n].rearrange("p (r w) -> p r w", w=W),
                       b=b_sb[bn]:
                    nc.scalar.activation(o, p, RELU, bias=b),
                "sa"))
        else:
            val = ("sa", op(
                "act",
                lambda o=dst[:, bass.ds(j * CH, n)], p=ps[:, :n],
                       b=b_sb[bn]:
                    nc.scalar.activation(o, p, RELU, bias=b),
                "sa"))
        epi_done[(ci, j)] = val
        last_slot_epi[(tidx, slot)] = val
        last_tensor_epi[tidx] = val

    wait("pe", "sde", SDE_ALL)
    wait("act", "sde", SDE_ALL)

    # group-0/1 prereqs first; tensor A gets the first 16 chunks so the
    # block phase (which starts on A) frees it early, B takes the tail 10
    conv_order = (
        [(0, j) for j in range(4)] + [(1, j) for j in range(2)]
        + [(2, j) for j in range(4)] + [(3, j) for j in range(4)]
        + [(4, j) for j in range(2)] + [(0, j) for j in range(4, 6)]
        + [(1, j) for j in range(2, 4)] + [(2, j) for j in range(4, 6)]
        + [(3, j) for j in range(4, 6)] + [(4, j) for j in range(2, 4)]
    )
    for idx, (ci, j) in enumerate(conv_order):
        if idx < 16:
            emit_conv_chunk(ci, j, 0, idx % 4)
        else:
            emit_conv_chunk(ci, j, 1, (idx - 16) % 4)

    # block-phase DVE prereqs (smask DMA, kpad/vpad border memsets)
    wait("dve", "sdw", SDW_ALL)
    wait("dve", "sg", MEMSETS)

    # ---- attention blocks: 4 groups of 4, software-pipelined ----
    sdone, expdone, muldone = {}, {}, {}
    qbdone, vndone, kndone, avdone, acdone = {}, {}, {}, {}, {}
    vtm, atm = {}, {}

    def grp_geom(grp):
        return grp // 2, grp % 2, grp % 2   # br, half, parity

    def st_gathers(grp):
        br, half, par = grp_geom(grp)
        r0 = br * BR
        # prereqs: q2 chunks for this block-row; v/k2 chunks for rows used
        wait("gp", *epi_done[(4, 2 * br + 1)])
        if grp >= 2:
            wait("gp", "sp", sdone[grp - 2])
        for i in range(4):
            c0 = (half * 4 + i) * BC
            qbdone[grp] = op(
                "gp",
                lambda d=qb4[:, par, i, :].rearrange("p (r w) -> p r w", r=BR),
                       s_=q[:, r0:r0 + BR, c0:c0 + BC]:
                    nc.gpsimd.tensor_copy(d, s_),
                "sg")
        wait("gp", *epi_done[(2, 3 if br == 0 else 5)])
        for i in range(4):
            c0 = (half * 4 + i) * BC
            if grp >= 2:
                wait("gp", f"sdtv{par}{i}", vtm[(grp - 2, i)])
            vndone[grp] = op(
                "gp",
                lambda d=vn4[:, par, i, 0:NN].rearrange(
                           "p (r w) -> p r w", r=NR),
                       s_=vpad[:, r0:r0 + NR, c0:c0 + NC_]:
                    nc.gpsimd.tensor_copy(d, s_),
                "sg")
        wait("dve", *epi_done[(3, 3 if br == 0 else 5)])
        if grp >= 2:
            wait("dve", "sp", sdone[grp - 2])
        for i in range(4):
            c0 = (half * 4 + i) * BC
            kndone[grp] = op(
                "dve",
                lambda d=kn4[:, par, i, 0:NN].rearrange(
                           "p (r w) -> p r w", r=NR),
                       s_=kpad[:, r0:r0 + NR, c0:c0 + NC_]:
                    nc.vector.tensor_copy(d, s_),
                "sv")

    def st_s(grp):
        br, half, par = grp_geom(grp)
        psX = PS[par]
        if last_tensor_epi[par]:
            wait("pe", *last_tensor_epi[par])
        wait("pe", "sg", qbdone[grp])
        wait("pe", "sv", kndone[grp])
        if grp >= 2:
            # S region previously read by exp (ACT); and the av region of
            # the SAME banks read by attn-copy — same-bank PE-write with a
            # concurrent ACT-read crashes the PSUM bank (P10), so wait for
            # the whole-bank readers, not just the S-region ones
            wait("pe", "sa", acdone[grp - 2])
        for i in range(4):
            sdone[(grp, i)] = sdone[grp] = op(
                "pe",
                lambda o=psX[:, i, 0:NN2], l=qb4[:, par, i, :],
                       r=kn4[:, par, i, :]:
                    nc.tensor.matmul(o, l, r, start=True, stop=True),
                "sp")

    def st_vtrans(grp):
        br, half, par = grp_geom(grp)
        wait("sync", "sg", vndone[grp])
        if grp >= 2:
            wait("sync", "sp", avdone[grp - 2])
        for i in range(4):
            vtm[(grp, i)] = op(
                "sync",
                lambda o=vt16[:, par, i], s_=vn4[:, par, i, :]:
                    nc.sync.dma_start(out=o, in_=s_, transpose=True),
                f"sdtv{par}{i}", 16)
        for i in range(4):
            wait("sync", f"sdtv{par}{i}", vtm[(grp, i)])

    def st_softmax(grp):
        br, half, par = grp_geom(grp)
        psX = PS[par]
        # ACT: exp straight off PSUM, per block
        if grp >= 2:
            wait("act", "sv", muldone[grp - 2])
        for i in range(4):
            wait("act", "sp", sdone[(grp, i)])
            expdone[(grp, i)] = expdone[grp] = op(
                "act",
                lambda o=e4[:, par, i, :], i_=psX[:, i, 0:NN2]:
                    nc.scalar.activation(o, i_, EXP, bias=eshift),
                "sa")
        # DVE: per-block chain am -> zsum -> +oob -> recip -> normalize,
        # emitted as a wavefront so same-engine RAW waits never stall
        amm, zrm, zam, rcm = {}, {}, {}, {}

        def dve_stage(s, i):
            if s == 0:
                wait("dve", "sa", expdone[(grp, i)])
                amm[i] = op(
                    "dve",
                    lambda o=am4[:, par, i, :], i0=e4[:, par, i, :],
                           i1=smask[:, br, :]:
                        nc.vector.tensor_mul(o, i0, i1),
                    "sv")
            elif s == 1:
                wait("dve", "sv", amm[i])
                zrm[i] = op(
                    "dve",
                    lambda o=z4[:, par, i:i + 1], i_=am4[:, par, i, :]:
                        nc.vector.reduce_sum(o, i_,
                                             axis=mybir.AxisListType.X),
                    "sv")
            elif s == 2:
                wait("dve", "sv", zrm[i])
                zam[i] = op(
                    "dve",
                    lambda o=zs4[:, par, i:i + 1], i_=z4[:, par, i:i + 1],
                           s_=oobc[:, br:br + 1]:
                        nc.vector.tensor_scalar_add(o, i_, s_),
                    "sv")
            elif s == 3:
                wait("dve", "sv", zam[i])
                rcm[i] = op(
                    "dve",
                    lambda o=rz4[:, par, i:i + 1], i_=zs4[:, par, i:i + 1]:
                        nc.vector.reciprocal(o, i_),
                    "sv")
            else:
                wait("dve", "sv", rcm[i])
                muldone[(grp, i)] = muldone[grp] = op(
                    "dve",
                    lambda o=a16[:, par, i, :], i_=am4[:, par, i, :],
                           s_=rz4[:, par, i:i + 1]:
                        nc.vector.tensor_scalar_mul(o, i_, s_),
                    "sv")

        for wv in range(9):
            for i in range(4):
                s = wv - i
                if 0 <= s < 5:
                    dve_stage(s, i)

    def st_atrans(grp):
        br, half, par = grp_geom(grp)
        # gate enqueue order: same sems are reused by group grp+2, and the
        # next enqueue must come after this group's AV consumed at16
        if grp >= 2:
            wait("sync", "sp", avdone[grp - 2])
        for i in range(4):
            wait("sync", "sv", muldone[(grp, i)])
            atm[(grp, i)] = op(
                "sync",
                lambda o=at16[:, par, i], s_=a16[:, par, i, :]:
                    nc.sync.dma_start(out=o, in_=s_, transpose=True),
                f"sdta{par}{i}", 16)
        for i in range(4):
            wait("sync", f"sdta{par}{i}", atm[(grp, i)])

    def st_av(grp):
        br, half, par = grp_geom(grp)
        psX = PS[par]
        if grp >= 2:
            wait("pe", "sa", acdone[grp - 2])
        for i in range(4):
            wait("pe", f"sdtv{par}{i}", vtm[(grp, i)])
            wait("pe", f"sdta{par}{i}", atm[(grp, i)])
            for ch in range(3):
                avdone[grp] = op(
                    "pe",
                    lambda o=psX[:, i, NN2:CH], l=vt16[:, par, i, ch, :],
                           r=at16[:, par, i, ch, :],
                           st=(ch == 0), sp_=(ch == 2):
                        nc.tensor.matmul(o, l, r, start=st, stop=sp_),
                    "sp")

    def st_accopy(grp):
        br, half, par = grp_geom(grp)
        psX = PS[par]
        r0 = br * BR
        wait("act", "sp", avdone[grp])
        acdone[grp] = op(
            "act",
            lambda o=attn[:, r0:r0 + BR,
                          half * 64:half * 64 + 64].rearrange(
                              "p r (b w) -> p b r w", w=BC),
                   i_=psX[:, :, NN2:CH].rearrange(
                       "p b (r w) -> p b r w", w=BC):
                nc.scalar.copy(o, i_),
            "sa")

    for grp in range(4):
        st_gathers(grp)
        if grp >= 2:
            st_accopy(grp - 2)
        st_s(grp)
        st_vtrans(grp)
        if grp >= 1:
            st_softmax(grp - 1)
            st_atrans(grp - 1)
            st_av(grp - 1)
    st_softmax(3)
    st_atrans(3)
    st_av(3)
    st_accopy(2)
    st_accopy(3)

    # ---- output conv ----
    attn_flat = attn.rearrange("p r w -> p (r w)")
    oc_done, yt_done = {}, {}
    wait("pe", "sa", acdone[3])
    for i in range(NPIX // CH):
        pq = i % 2
        ps = PS[pq][:, 0, :]
        if i >= 2:
            wait("pe", "sa", yt_done[i - 2])
        op("pe",
           lambda o=ps, l=w_sb["wat"], r=attn_flat[:, bass.ts(i, CH)]:
               nc.tensor.matmul(o, l, r, start=True, stop=False),
           "sp")
        oc_done[i] = op(
            "pe",
            lambda o=ps, l=w_sb["wxt"],
                   r=xs[:, bass.ds(PAD * W + i * CH, CH)]:
                nc.tensor.matmul(o, l, r, start=False, stop=True),
            "sp")
        wait("act", "sp", oc_done[i])
        if i >= 2:
            wait("act", f"sdout{pq}", 16 * (i // 2))
        yt_done[i] = op(
            "act",
            lambda o=yt[:, pq, :], i_=ps, b=b_sb["bo"]:
                nc.scalar.activation(o, i_, IDENT, bias=b),
            "sa")
        wait("sync", "sa", yt_done[i])
        op("sync",
           lambda o=y_d[:, bass.ts(i, CH)], i_=yt[:, pq, :]:
               nc.sync.dma_start(out=o, in_=i_),
           f"sdout{pq}", 16)

    # ---- tail: wait everything before the final barrier ----
    wait("sync", "sp", cnt["sp"])
    wait("sync", "sa", cnt["sa"])
    wait("sync", "sv", cnt["sv"])
    wait("sync", "sg", cnt["sg"])
    wait("sync", "sdout0", cnt["sdout0"])
    wait("sync", "sdout1", cnt["sdout1"])
    wait("sync", "sde", SDE_ALL)
    wait("sync", "sdw", SDW_ALL)
    for j in range(6):
        wait("sync", f"sdx{j}", cnt[f"sdx{j}"])
    for p_ in range(2):
        for i_ in range(4):
            wait("sync", f"sdta{p_}{i_}", cnt[f"sdta{p_}{i_}"])
            wait("sync", f"sdtv{p_}{i_}", cnt[f"sdtv{p_}{i_}"])

    # ---- emit ----
    def run(eng_name, eng_obj):
        hwm = {}
        for item in plan[eng_name]:
            if item[0] == "w":
                _, s_, v = item
                if hwm.get(s_, 0) >= v:
                    continue
                hwm[s_] = v
                eng_obj.wait_ge(sems[s_], v)
            else:
                _, fn, s_, inc = item
                inst = fn()
                if s_:
                    inst.then_inc(sems[s_], inc)

    with nc.Block() as block:
        @block.sync
        def _(e):
            run("sync", e)

        @block.tensor
        def _(e):
            run("pe", e)

        @block.scalar
        def _(e):
            run("act", e)

        @block.vector
        def _(e):
            run("dve", e)

        @block.gpsimd
        def _(e):
            run("gp", e)

    with nc.Block() as block2:
        @block2.sync
        def _(e):
            for n in sem_names:
                nc.sync.sem_clear(sems[n])

    return nc


_PROGRAM = None


def _host_inputs(x, w_q1, s_q1, b_q1, w_q2, s_q2, b_q2,
                 w_k1, s_k1, b_k1, w_k2, s_k2, b_k2,
                 w_v, s_v, b_v, w_o, s_o, b_o):
    """Per-core input dicts (numpy) for the SPMD program."""
    def foldT(w, s):
        return np.ascontiguousarray((s[:, None] * w).T.astype(ml_dtypes.bfloat16))

    wq1t, wq2t = foldT(w_q1, s_q1), foldT(w_q2, s_q2)
    wk1t, wk2t = foldT(w_k1, s_k1), foldT(w_k2, s_k2)
    wvt = foldT(w_v, s_v)
    wo = s_o[:, None] * w_o
    wat = np.ascontiguousarray(wo[:, :C].T.astype(ml_dtypes.bfloat16))
    wxt = np.ascontiguousarray(wo[:, C:].T.astype(ml_dtypes.bfloat16))

    col = lambda b: np.ascontiguousarray(b.astype(np.float32)[:, None])

    # window-validity over the 14x22 neighborhood, per block pixel
    valid = np.zeros((BR * BC, NR, NC_), bool)
    for r in range(BR):
        for c in range(BC):
            p = r * BC + c
            valid[p, r:r + 7, c:c + 7] = True

    X = np.asarray(x, np.float32).reshape(C, H, W)
    wall = np.concatenate(
        [wq1t, wq2t, wk1t, wk2t, wvt, wat, wxt,
         np.eye(C, dtype=ml_dtypes.bfloat16)], axis=1)
    shared = dict(wall=np.ascontiguousarray(wall))

    e16v = np.float32(np.exp(EXP_SHIFT))
    in_maps = []
    for core in range(NCORES):
        h0 = core * RPC
        xsb = np.zeros((C, HALO, W), np.float32)
        lo, hi = h0 - PAD, h0 + RPC + PAD
        slo, shi = max(lo, 0), min(hi, H)
        xsb[:, slo - lo:shi - lo] = X[:, slo:shi]

        # per-block-row multiplicative 0/1 mask (0 for off-window, OOB-row,
        # and pad cols) and out-of-image-row Z compensation
        smask = np.zeros((NBR, BR * BC, NN2), np.float32)
        oobc = np.zeros((NBR, BR * BC), np.float32)
        for brr in range(NBR):
            rowok = np.array([0 <= h0 + brr * BR + ri - PAD < H
                              for ri in range(NR)])
            m = (valid & rowok[None, :, None]).astype(np.float32)
            smask[brr, :, :NN] = m.reshape(BR * BC, NN)
            for r in range(BR):
                n_oob = sum(1 for i in range(7)
                            if not (0 <= h0 + brr * BR + r - PAD + i < H))
                oobc[brr, r * BC:(r + 1) * BC] = 7 * n_oob * e16v
        m = dict(shared)
        m["xs"] = np.ascontiguousarray(
            xsb.reshape(C, NHALO).astype(ml_dtypes.bfloat16))
        m["smask"] = np.ascontiguousarray(
            smask.transpose(1, 0, 2).astype(ml_dtypes.bfloat16))
        m["ball"] = np.ascontiguousarray(np.concatenate(
            [col(b_q1), col(b_q2), col(b_k1), col(b_k2), col(b_v),
             col(b_o), oobc.T.astype(np.float32)], axis=1))
        in_maps.append(m)
    return in_maps


def kernel(**inputs):
    global _PROGRAM
    if _PROGRAM is None:
        _PROGRAM = _build_program()
    in_maps = _host_inputs(**{k: np.asarray(v) for k, v in inputs.items()})
    res = run_bass_kernel_spmd(_PROGRAM, in_maps, core_ids=list(range(NCORES)))
    stripes = [np.asarray(r["y"]).reshape(C, RPC, W) for r in res.results]
    return np.concatenate(stripes, axis=1).reshape(1, C, H, W)


if __name__ == "__main__":
    rng = np.random.default_rng(0)
    fake = {"x": rng.standard_normal((1, C, H, W), np.float32)}
    for n in ("q1", "q2", "k1", "k2", "v", "o"):
        cin = 2 * C if n == "o" else C
        fake["w_" + n] = rng.standard_normal((C, cin), np.float32) / np.sqrt(cin)
        fake["s_" + n] = rng.uniform(0.5, 1.5, C).astype(np.float32)
        fake["b_" + n] = (rng.standard_normal(C) * 0.1).astype(np.float32)
    out = kernel(**fake)
    print("kernel output", out.shape, out.dtype)



# revision 3
# speedup vs baseline: 1.0681x; 1.0681x over previous
"""Trainium2 Bass kernel for ModalitySpecificLocalSelfAttention (7x7 window).

v2 strategy (8 NeuronCores, spatial-parallel over H, column-major layout):
  - Each core owns 16 image rows; spatial tensors are stored COLUMN-major
    (idx = col*rows + row) over a padded halo rect of 22 rows x 134 cols.
    Full-height pixel blocks (16 rows x 8 cols = 128 px) then have
    CONTIGUOUS neighborhoods (14 cols x 22 rows = 308 <= 384 wide), so the
    S matmul needs no gather: lhsT = q[:, 128*bc:...], rhs = kpad[:, 176*bc:+384].
  - The 7x7 window mask is an EXACT rank-24 additive matmul (row-indicator
    x col-window factorization, -100 off-window), accumulated into the S
    PSUM. exp(S + mask - 16) then reproduces zero-pad softmax semantics;
    out-of-image rows are compensated in Z via a per-partition constant.
  - Softmax: exp on ACT (PSUM->SBUF, batched 2 blocks), Z-reduce / +oob /
    recip / normalize on DVE (per-partition scalars only).
  - W_a of the output conv is folded into the V path (wav = (s_o W_a) @ v),
    so attention aggregation directly accumulates the output: per block
    y += wavt_chunk.T @ at_chunk in PSUM, plus W_x @ x and a rank-1 bias
    matmul. y is DMA'd straight from PSUM.
  - Transposes of a and wav are per-block XBAR DMAs [128,384] -> 3 tiles
    of [128,128] (no gathers, no PSUM copies).
"""

import sys

for _p in ("/opt/trn_rl_repo", "/root/.axon_site/_ro/trn_rl_repo"):
    if _p not in sys.path:
        sys.path.append(_p)

import ml_dtypes
import numpy as np

import concourse.bass as bass
from concourse import mybir
from concourse.bass_utils import run_bass_kernel_spmd

F32 = mybir.dt.float32
BF16 = mybir.dt.bfloat16

C = 128
H = 128
W = 128
NCORES = 8
RPC = H // NCORES          # 16 rows per core
PAD = 3
HR = RPC + 2 * PAD         # 22 halo rows
WPC = W + 2 * PAD          # 134 padded cols
RECT = HR * WPC            # 2948
RECTA = 3072               # allocated (24 x 128) for XBAR alignment
OWN = RPC * W              # 2048 owned pixels
NB = 16                    # blocks of 8 image cols x 16 rows
BPX = 128                  # pixels per block
NW = 384                   # widened neighborhood (real 308 = 14*22)
WSTRIDE = 8 * HR           # 176: rect offset between consecutive blocks
MR = 24                    # mask rank (8 col + 16 row indicators)
CH = 512                   # conv chunk (one PSUM bank of f32)
ESH = -16.0                # exp shift
MBIG = -100.0              # additive off-window mask

NKC = RECTA // CH          # 6 chunks for rect convs
NQC = OWN // CH            # 4 chunks for owned convs


def _build_program():
    nc = bass.Bass("TRN2", target_bir_lowering=False, debug=False)

    # ---- DRAM I/O ----
    xs_d = nc.dram_tensor("xs", [C, RECTA], BF16, kind="ExternalInput").ap()
    wall_d = nc.dram_tensor("wall", [C, 7 * C], BF16, kind="ExternalInput").ap()
    lr_d = nc.dram_tensor("lr", [MR, BPX + NW], BF16, kind="ExternalInput").ap()
    sm_d = nc.dram_tensor("sm", [1, C + CH], BF16, kind="ExternalInput").ap()
    bias_d = nc.dram_tensor("bias", [C, 6], F32, kind="ExternalInput").ap()
    y_d = nc.dram_tensor("y", [C, OWN], F32, kind="ExternalOutput").ap()

    # ---- SBUF ----
    sb = lambda name, shape, dt: nc.alloc_sbuf_tensor(name, list(shape), dt).ap()
    xs = sb("xs_sb", [C, RECTA], BF16)
    k1 = sb("k1_sb", [C, RECTA], BF16)
    kpad = sb("kpad_sb", [C, RECTA], BF16)
    vpad = sb("vpad_sb", [C, RECTA], BF16)
    wav = sb("wav_sb", [C, RECTA], BF16)
    q1 = sb("q1_sb", [C, OWN], BF16)
    q = sb("q_sb", [C, OWN], BF16)
    wall = sb("wall_sb", [C, 7 * C], BF16)
    lr = sb("lr_sb", [MR, BPX + NW], BF16)
    sm = sb("sm_sb", [1, C + CH], BF16)
    bias = sb("bias_sb", [C, 6], F32)
    ae = sb("ae_sb", [C, NB, NW], BF16)      # exp(S+mask-16)
    aa = sb("aa_sb", [C, NB, NW], BF16)      # normalized attention
    at = sb("at_sb", [C, 3 * NB, C], BF16)   # transposed aa, 3 tiles/block
    wavt = sb("wavt_sb", [C, 3 * NB, C], BF16)
    zz = sb("zz_sb", [C, 4], F32)
    rz = sb("rz_sb", [C, 4], F32)
    yt = sb("yt_sb", [C, 2, CH], F32)
    esh = sb("esh_sb", [C, 1], F32)

    W_NAMES = ("wk1t", "wq1t", "wk2t", "wvt", "wq2t", "wxt", "wavw")
    w_sb = {n: wall[:, bass.ts(i, C)] for i, n in enumerate(W_NAMES)}
    Lm = lr[:, 0:BPX]
    Rm = lr[:, BPX:BPX + NW]
    bo_row = sm[:, 0:C]
    ones_row = sm[:, C:C + CH]
    b_col = {n: bias[:, i:i + 1]
             for i, n in enumerate(("bk1", "bq1", "bk2", "bv", "bq2"))}
    oobc = bias[:, 5:6]

    # ---- PSUM: two S pair-tensors (2 banks each) + two y banks ----
    ps_sp = [nc.alloc_psum_tensor(f"ps_s{i}", [C, 2, CH], F32).ap()
             for i in range(2)]
    ps_yp = [nc.alloc_psum_tensor(f"ps_y{i}", [C, CH], F32).ap()
             for i in range(2)]

    def s_bank(bc):  # bank for block bc
        return ps_sp[(bc // 2) % 2][:, bc % 2, :]

    def conv_slot(i):
        return ps_sp[i // 2][:, i % 2, :] if i < 4 else ps_yp[i - 4]

    # ---- semaphores ----
    sem_names = (["sdin", "sp", "sa", "sv", "sg", "syd0", "syd1"]
                 + [f"swt{j}" for j in range(NB)]
                 + [f"sax{j}" for j in range(NB)])
    sems = {n: nc.alloc_semaphore(n) for n in sem_names}

    ENGS = ("sync", "pe", "act", "dve", "gp")
    plan = {e: [] for e in ENGS}
    cnt = {n: 0 for n in sem_names}

    def op(eng, fn, sem, inc=1):
        plan[eng].append(("op", fn, sem, inc))
        if sem:
            cnt[sem] += inc
            return (sem, cnt[sem])
        return None

    def wait(eng, mark):
        if mark:
            s_, v = mark
            if v > 0:
                plan[eng].append(("w", s_, v))

    RELU = mybir.ActivationFunctionType.Relu
    EXP = mybir.ActivationFunctionType.Exp
    ADD = mybir.AluOpType.add
    MAX = mybir.AluOpType.max

    # ================= input DMAs =================
    for dst, srcd in ((xs, xs_d), (wall, wall_d), (lr, lr_d), (sm, sm_d),
                      (bias, bias_d)):
        SDIN = op("sync", lambda d=dst, s=srcd: nc.sync.dma_start(out=d, in_=s),
                  "sdin", 16)

    ESHM = op("dve", lambda: nc.vector.memset(esh, ESH), "sv")

    # ================= convs =================
    # (name, weights, rhs_fn(j), nchunks, dst, bias_or_None, epi_engine)
    xsr = xs[:, 0:RECT].rearrange("p (c r) -> p c r", r=HR)

    def rect_rhs(src):
        return lambda j: src[:, bass.ts(j, CH)]

    def own_rhs(src, rect):
        if rect:
            return lambda j: xsr[:, PAD + 32 * j:PAD + 32 * (j + 1), PAD:PAD + RPC]
        return lambda j: src[:, bass.ts(j, CH)]

    # epilogue engine rotation for relu+bias epis
    conv_list = [
        ("k1", "wk1t", rect_rhs(xs), NKC, k1, "bk1"),
        ("q1", "wq1t", own_rhs(xs, True), NQC, q1, "bq1"),
        ("k2", "wk2t", rect_rhs(k1), NKC, kpad, "bk2"),
        ("v", "wvt", rect_rhs(xs), NKC, vpad, "bv"),
        ("q2", "wq2t", rect_rhs(q1), NQC, q, "bq2"),
        ("wav", "wavw", rect_rhs(vpad), NKC, wav, None),
    ]
    ci_of = {c[0]: i for i, c in enumerate(conv_list)}
    src_of = {"k2": "k1", "q2": "q1", "wav": "v"}

    epi_done = {}        # (cname, j) -> mark
    last_slot_user = {}  # slot -> mark (epi that freed it)
    slot_i = [0]

    EPI_ROT = ("act", "dve")
    epi_i = [0]

    def emit_conv_chunk(cname, j):
        ci = ci_of[cname]
        _, wn, rhsf, nch, dst, bn = conv_list[ci]
        slot = slot_i[0] % 6
        slot_i[0] += 1
        ps = conv_slot(slot)
        if cname in src_of:
            wait("pe", epi_done.get((src_of[cname], j)))
        if cname == "wav":
            for mk_ in mems_v:
                wait("pe", mk_)
        wait("pe", last_slot_user.get(slot))
        mm = op("pe", lambda p=ps, w_=w_sb[wn], r=rhsf(j):
                nc.tensor.matmul(p, w_, r, start=True, stop=True), "sp")
        eng = EPI_ROT[epi_i[0] % len(EPI_ROT)] if bn else ("act", "dve")[j % 2]
        epi_i[0] += bn is not None
        wait(eng, mm)
        if bn is not None:
            if eng == "act":
                mk = op(eng, lambda o=dst[:, bass.ts(j, CH)], p=ps, b=b_col[bn]:
                        nc.scalar.activation(o, p, RELU, bias=b), "sa")
            else:
                e_obj = nc.vector if eng == "dve" else nc.gpsimd
                mk = op(eng, lambda o=dst[:, bass.ts(j, CH)], p=ps, b=b_col[bn],
                        eo=e_obj: eo.tensor_scalar(o, p, b, 0.0, ADD, MAX),
                        "sv" if eng == "dve" else "sg")
        elif eng == "act":
            mk = op("act", lambda o=dst[:, bass.ts(j, CH)], p=ps:
                    nc.scalar.copy(o, p), "sa")
        else:
            mk = op("dve", lambda o=dst[:, bass.ts(j, CH)], p=ps:
                    nc.vector.tensor_copy(o, p), "sv")
        epi_done[(cname, j)] = mk
        last_slot_user[slot] = mk

    wait("pe", SDIN)
    # interleave: k1 fully, q1, then k2/v, q2, wav
    mems_k, mems_v = [], []
    for j in range(NKC):
        emit_conv_chunk("k1", j)
    for j in range(NQC):
        emit_conv_chunk("q1", j)
    for j in range(NKC):
        emit_conv_chunk("k2", j)
        emit_conv_chunk("v", j)

    # pad-col memsets AFTER the epilogues that write garbage into pads
    kpr = kpad[:, 0:RECT].rearrange("p (c r) -> p c r", r=HR)
    vpr = vpad[:, 0:RECT].rearrange("p (c r) -> p c r", r=HR)
    for t, lst, nm in ((kpr, mems_k, "k2"), (vpr, mems_v, "v")):
        wait("gp", epi_done[(nm, 0)])
        lst.append(op("gp", lambda tf=t[:, 0:PAD, :]:
                      nc.gpsimd.memset(tf, 0.0), "sg"))
        wait("gp", epi_done[(nm, NKC - 1)])
        lst.append(op("gp", lambda tf=t[:, PAD + W:WPC, :]:
                      nc.gpsimd.memset(tf, 0.0), "sg"))

    for j in range(NQC):
        emit_conv_chunk("q2", j)
    for j in range(NKC):
        emit_conv_chunk("wav", j)

    # ================= wavt XBAR transposes =================
    wvt_mark = {}
    for bc in range(NB):
        hi_chunk = (WSTRIDE * bc + NW - 1) // CH
        wait("sync", epi_done[("wav", hi_chunk)])
        wvt_mark[bc] = op(
            "sync",
            lambda o=wavt[:, 3 * bc:3 * bc + 3, :],
                   i_=wav[:, bass.ds(WSTRIDE * bc, NW)]:
                nc.sync.dma_start(out=o, in_=i_, transpose=True),
            f"swt{bc}", 16)

    # ================= attention blocks =================
    s_done, exp_done, norm_done, ax_mark, grp_done = {}, {}, {}, {}, {}
    ydma = {}

    def st_s(bc):
        slot = bc % 4
        ps = s_bank(bc)[:, 0:NW]
        wait("pe", epi_done[("q2", bc // 4)])
        wait("pe", epi_done[("k2", min((WSTRIDE * bc + NW - 1) // CH, NKC - 1))])
        for mk_ in mems_k:
            wait("pe", mk_)
        if bc >= 4:
            wait("pe", exp_done[(bc - 4) // 2])
        else:
            wait("pe", last_slot_user.get(slot))
        op("pe", lambda o=ps, l=q[:, bass.ts(bc, BPX)],
                 r=kpad[:, bass.ds(WSTRIDE * bc, NW)]:
           nc.tensor.matmul(o, l, r, start=True, stop=False), "sp")
        s_done[bc] = op("pe", lambda o=ps, l=Lm, r=Rm:
                        nc.tensor.matmul(o, l, r, start=False, stop=True), "sp")

    def st_softmax_pair(i):  # blocks 2i, 2i+1
        bc = 2 * i
        b0 = bc % 4
        wait("act", s_done[bc + 1])
        wait("act", ESHM)
        if i >= 2:
            wait("dve", norm_done[2 * (i - 2) + 1])
        exp_done[i] = op(
            "act",
            lambda o=ae[:, bc:bc + 2, :], i_=ps_sp[i % 2][:, :, 0:NW]:
                nc.scalar.activation(o, i_, EXP, bias=esh), "sa")
        wait("dve", exp_done[i])
        zr = op("dve", lambda o=zz[:, b0:b0 + 2], i_=ae[:, bc:bc + 2, :]:
                nc.vector.reduce_sum(o, i_, axis=mybir.AxisListType.X), "sv")
        wait("dve", zr)
        za = op("dve", lambda o=zz[:, b0:b0 + 2], i_=zz[:, b0:b0 + 2], s_=oobc:
                nc.vector.tensor_scalar_add(o, i_, s_), "sv")
        wait("dve", za)
        rc = op("dve", lambda o=rz[:, b0:b0 + 2], i_=zz[:, b0:b0 + 2]:
                nc.vector.reciprocal(o, i_), "sv")
        wait("dve", rc)
        wait("gp", rc)
        norm_done[bc] = op(
            "dve",
            lambda o=aa[:, bc, :], i_=ae[:, bc, :], s_=rz[:, b0:b0 + 1]:
                nc.vector.tensor_scalar_mul(o, i_, s_), "sv")
        norm_done[bc + 1] = op(
            "gp",
            lambda o=aa[:, bc + 1, :], i_=ae[:, bc + 1, :],
                   s_=rz[:, b0 + 1:b0 + 2]:
                nc.gpsimd.tensor_scalar_mul(o, i_, s_), "sg")

    def st_ax(bc):
        wait("sync", norm_done[bc])
        ax_mark[bc] = op(
            "sync",
            lambda o=at[:, 3 * bc:3 * bc + 3, :], i_=aa[:, bc, :]:
                nc.sync.dma_start(out=o, in_=i_, transpose=True),
            f"sax{bc}", 16)

    def st_group(g):  # Wx + bias + AV for blocks 4g..4g+3
        pq = g % 2
        ps = ps_yp[pq]
        if g >= 2:
            wait("pe", ycopy[g - 2])
        else:
            wait("pe", last_slot_user.get(4 + pq))
        op("pe", lambda o=ps, l=w_sb["wxt"],
                 r=xsr[:, PAD + 32 * g:PAD + 32 * (g + 1), PAD:PAD + RPC]:
           nc.tensor.matmul(o, l, r, start=True, stop=False,
                            skip_group_check=True), "sp")
        op("pe", lambda o=ps, l=bo_row, r=ones_row:
           nc.tensor.matmul(o, l, r, start=False, stop=False,
                            skip_group_check=True), "sp")
        last = None
        for i in range(4):
            bc = 4 * g + i
            wait("pe", ax_mark[bc])
            wait("pe", wvt_mark[bc])
            for t in range(3):
                fin = (i == 3 and t == 2)
                last = op(
                    "pe",
                    lambda o=ps[:, bass.ts(i, BPX)],
                           l=wavt[:, 3 * bc + t, :], r=at[:, 3 * bc + t, :],
                           sp_=fin:
                        nc.tensor.matmul(o, l, r, start=False, stop=sp_,
                                         skip_group_check=True), "sp")
        grp_done[g] = last

    for bc in range(NB):
        st_s(bc)
        if bc % 2 == 1:
            st_softmax_pair(bc // 2)
            st_ax(bc - 1)
            st_ax(bc)

    ycopy = {}
    for g in range(4):
        st_group(g)
        ceng = ("act", "dve")[g % 2]
        wait(ceng, grp_done[g])
        if g >= 2:
            wait(ceng, ydma[g - 2])
        if ceng == "act":
            ycopy[g] = op("act", lambda o=yt[:, g % 2, :], i_=ps_yp[g % 2]:
                          nc.scalar.copy(o, i_), "sa")
        else:
            ycopy[g] = op("dve", lambda o=yt[:, g % 2, :], i_=ps_yp[g % 2]:
                          nc.vector.tensor_copy(o, i_), "sv")
        wait("sync", ycopy[g])
        ydma[g] = op(
            "sync",
            lambda o=y_d[:, bass.ts(g, CH)], i_=yt[:, g % 2, :]:
                nc.sync.dma_start(out=o, in_=i_),
            f"syd{g % 2}", 16)

    # ---- tail barrier ----
    for s_ in ("sp", "sa", "sv", "sg"):
        wait("sync", (s_, cnt[s_]))
    wait("sync", ("sdin", cnt["sdin"]))
    for j in range(NB):
        wait("sync", (f"swt{j}", cnt[f"swt{j}"]))
        wait("sync", (f"sax{j}", cnt[f"sax{j}"]))
    wait("sync", ("syd0", cnt["syd0"]))
    wait("sync", ("syd1", cnt["syd1"]))

    # ---- emit ----
    def run(eng_name, eng_obj):
        hwm = {}
        for item in plan[eng_name]:
            if item[0] == "w":
                _, s_, v = item
                if hwm.get(s_, 0) >= v:
                    continue
                hwm[s_] = v
                eng_obj.wait_ge(sems[s_], v)
            else:
                _, fn, s_, inc = item
                inst = fn()
                if s_:
                    inst.then_inc(sems[s_], inc)

    with nc.Block() as block:
        @block.sync
        def _(e):
            run("sync", e)

        @block.tensor
        def _(e):
            run("pe", e)

        @block.scalar
        def _(e):
            run("act", e)

        @block.vector
        def _(e):
            run("dve", e)

        @block.gpsimd
        def _(e):
            run("gp", e)

    with nc.Block() as block2:
        @block2.sync
        def _(e):
            for n in sem_names:
                nc.sync.sem_clear(sems[n])

    return nc


_PROGRAM = None


def _host_inputs(x, w_q1, s_q1, b_q1, w_q2, s_q2, b_q2,
                 w_k1, s_k1, b_k1, w_k2, s_k2, b_k2,
                 w_v, s_v, b_v, w_o, s_o, b_o):
    def foldT(w, s):
        return np.ascontiguousarray((s[:, None] * w).T.astype(ml_dtypes.bfloat16))

    wq1t, wq2t = foldT(w_q1, s_q1), foldT(w_q2, s_q2)
    wk1t, wk2t = foldT(w_k1, s_k1), foldT(w_k2, s_k2)
    wvt = foldT(w_v, s_v)
    wo = s_o[:, None] * np.asarray(w_o, np.float32)
    wat = np.ascontiguousarray(wo[:, :C].T.astype(ml_dtypes.bfloat16))
    wxt = np.ascontiguousarray(wo[:, C:].T.astype(ml_dtypes.bfloat16))
    # wav conv weight: wavw[c_in, c_out] so that wav = wat.T @ v
    wall = np.concatenate([wk1t, wq1t, wk2t, wvt, wq2t, wxt, wat], axis=1)

    # mask factors: L [24, 128] pixel indicators, R [24, 384] window terms
    L = np.zeros((MR, BPX), np.float32)
    for p in range(BPX):
        pc, pr = p // RPC, p % RPC
        L[pc, p] = 1.0
        L[8 + pr, p] = 1.0
    e16 = np.float32(np.exp(ESH))

    sm = np.zeros((1, C + CH), np.float32)
    sm[0, :C] = b_o
    sm[0, C:] = 1.0

    X = np.asarray(x, np.float32).reshape(C, H, W)
    shared = dict(
        wall=np.ascontiguousarray(wall),
        sm=np.ascontiguousarray(sm.astype(ml_dtypes.bfloat16)),
    )

    col = lambda b: b.astype(np.float32)[:, None]
    in_maps = []
    for core in range(NCORES):
        h0 = core * RPC
        # xs rect: [22 rows, 134 cols] col-major, zeros outside image
        rect = np.zeros((C, HR, WPC), np.float32)
        lo, hi = h0 - PAD, h0 + RPC + PAD
        slo, shi = max(lo, 0), min(hi, H)
        rect[:, slo - lo:shi - lo, PAD:PAD + W] = X[:, slo:shi, :]
        xs_cm = np.zeros((C, RECTA), np.float32)
        xs_cm[:, :RECT] = rect.transpose(0, 2, 1).reshape(C, RECT)

        rowok = np.array([0 <= h0 + nr - PAD < H for nr in range(HR)])
        R = np.zeros((MR, NW), np.float32)
        for n in range(NW):
            ncol, nrow = n // HR, n % HR
            for k in range(8):           # pixel col pc = k: window cols k..k+6
                if not (k <= ncol <= k + 6):
                    R[k, n] = MBIG
            for j in range(RPC):         # pixel row pr = j: halo rows j..j+6
                if not (j <= nrow <= j + 6 and rowok[nrow]):
                    R[8 + j, n] = MBIG
        lrm = np.concatenate([L, R], axis=1)

        oob = np.zeros((C, 1), np.float32)
        for p in range(BPX):
            pr = p % RPC
            n_oob = sum(1 for i in range(7) if not (0 <= h0 + pr - PAD + i < H))
            oob[p, 0] = 7 * n_oob * e16

        biases = np.concatenate(
            [col(b_k1), col(b_q1), col(b_k2), col(b_v), col(b_q2), oob], axis=1)

        m = dict(shared)
        m["xs"] = np.ascontiguousarray(xs_cm.astype(ml_dtypes.bfloat16))
        m["lr"] = np.ascontiguousarray(lrm.astype(ml_dtypes.bfloat16))
        m["bias"] = np.ascontiguousarray(biases.astype(np.float32))
        in_maps.append(m)
    return in_maps


def kernel(**inputs):
    global _PROGRAM
    if _PROGRAM is None:
        _PROGRAM = _build_program()
    in_maps = _host_inputs(**{k: np.asarray(v) for k, v in inputs.items()})
    res = run_bass_kernel_spmd(_PROGRAM, in_maps, core_ids=list(range(NCORES)))
    stripes = []
    for r in res.results:
        y = np.asarray(r["y"]).reshape(C, W, RPC)      # col-major -> [C, col, row]
        stripes.append(y.transpose(0, 2, 1))           # [C, 16, 128]
    return np.concatenate(stripes, axis=1).reshape(1, C, H, W)


if __name__ == "__main__":
    rng = np.random.default_rng(0)
    fake = {"x": rng.standard_normal((1, C, H, W), np.float32)}
    for n in ("q1", "q2", "k1", "k2", "v", "o"):
        cin = 2 * C if n == "o" else C
        fake["w_" + n] = (rng.standard_normal((C, cin)) / np.sqrt(cin)).astype(np.float32)
        fake["s_" + n] = rng.uniform(0.5, 1.5, C).astype(np.float32)
        fake["b_" + n] = (rng.standard_normal(C) * 0.1).astype(np.float32)
    out = kernel(**fake)
    print("kernel output", out.shape, out.dtype)


# revision 4
# speedup vs baseline: 1.4148x; 1.3247x over previous
"""Trainium2 Bass kernel for ModalitySpecificLocalSelfAttention (7x7 window).

v3: col-major layout (see v2 docstring) with per-instruction overheads
minimized after hardware profiling:
  - conv epilogues run on PAIRS of PSUM banks ([C,1024] per op), ACT mostly;
  - softmax stats at GROUP granularity (4 blocks): one reduce (Z includes a
    pre-written oob column), one reciprocal, one stride-0-broadcast
    tensor_mul normalize;
  - all 32 per-block XBAR transposes collapsed to 4 strided gather DMAs +
    4 batched wavt XBARs (gpsimd-issued) and 4 batched a-XBARs (sync);
  - S/mask matmuls at width 308 (tails handled by one-time memsets).
"""

import sys

for _p in ("/opt/trn_rl_repo", "/root/.axon_site/_ro/trn_rl_repo"):
    if _p not in sys.path:
        sys.path.append(_p)

import ml_dtypes
import numpy as np

import concourse.bass as bass
from concourse import mybir
from concourse.bass_utils import run_bass_kernel_spmd

F32 = mybir.dt.float32
BF16 = mybir.dt.bfloat16

C = 128
H = 128
W = 128
NCORES = 8
RPC = H // NCORES          # 16 rows per core
PAD = 3
HR = RPC + 2 * PAD         # 22 halo rows
WPC = W + 2 * PAD          # 134 padded cols
RECT = HR * WPC            # 2948
RECTA = 3072               # allocated (24 x 128)
OWN = RPC * W              # 2048 owned pixels
NB = 16                    # blocks of 8 image cols x 16 rows
BPX = 128                  # pixels per block
NW = 384                   # padded neighborhood width (real 308)
NWR = 308                  # real neighborhood (14 cols x 22 rows)
WSTRIDE = 8 * HR           # 176
MR = 24                    # mask rank
CH = 512                   # conv chunk (one PSUM bank of f32)
ESH = -16.0
MBIG = -100.0

NKC = RECTA // CH          # 6 chunks (3 pairs) rect convs
NQC = OWN // CH            # 4 chunks (2 pairs) owned convs


def _build_program():
    nc = bass.Bass("TRN2", target_bir_lowering=False, debug=False)

    # ---- DRAM I/O ----
    xs_d = nc.dram_tensor("xs", [C, RECTA], BF16, kind="ExternalInput").ap()
    wall_d = nc.dram_tensor("wall", [C, 7 * C], BF16, kind="ExternalInput").ap()
    lr_d = nc.dram_tensor("lr", [MR, BPX + NWR], BF16, kind="ExternalInput").ap()
    sm_d = nc.dram_tensor("sm", [1, C + CH], BF16, kind="ExternalInput").ap()
    bias_d = nc.dram_tensor("bias", [C, 6], F32, kind="ExternalInput").ap()
    y_d = nc.dram_tensor("y", [C, OWN], F32, kind="ExternalOutput").ap()

    # ---- SBUF ----
    sb = lambda name, shape, dt: nc.alloc_sbuf_tensor(name, list(shape), dt).ap()
    xs = sb("xs_sb", [C, RECTA], BF16)
    k1 = sb("k1_sb", [C, RECTA], BF16)
    kpad = sb("kpad_sb", [C, RECTA], BF16)
    vpad = sb("vpad_sb", [C, RECTA], BF16)
    wav = sb("wav_sb", [C, RECTA], BF16)
    q1 = sb("q1_sb", [C, OWN], BF16)
    q = sb("q_sb", [C, OWN], BF16)
    wall = sb("wall_sb", [C, 7 * C], BF16)
    lr = sb("lr_sb", [MR, BPX + NWR], BF16)
    sm = sb("sm_sb", [1, C + CH], BF16)
    bias = sb("bias_sb", [C, 6], F32)
    ae = sb("ae_sb", [C, NB, NW], BF16)      # exp; col 308 = oob Z term
    aa = sb("aa_sb", [C, NB, NW], BF16)      # normalized attention
    at = sb("at_sb", [C, 3 * NB, C], BF16)
    wavt = sb("wavt_sb", [C, 3 * NB, C], BF16)
    wavg = sb("wavg_sb", [C, 2, 4, NW], BF16)  # gathered wav windows (parity)
    zz = sb("zz_sb", [C, 8], F32)
    rz = sb("rz_sb", [C, 8], F32)
    yt = sb("yt_sb", [C, 2, CH], F32)
    esh = sb("esh_sb", [C, 1], F32)
    escr = sb("escr_sb", [C, 2 * CH], BF16)

    W_NAMES = ("wk1t", "wq1t", "wk2t", "wvt", "wq2t", "wxt", "wavw")
    w_sb = {n: wall[:, bass.ts(i, C)] for i, n in enumerate(W_NAMES)}
    Lm = lr[:, 0:BPX]
    Rm = lr[:, BPX:BPX + NWR]
    bo_row = sm[:, 0:C]
    ones_row = sm[:, C:C + CH]
    b_col = {n: bias[:, i:i + 1]
             for i, n in enumerate(("bk1", "bq1", "bk2", "bv", "bq2"))}
    oobc = bias[:, 5:6]

    # ---- PSUM: 3 conv/S pair-tensors (6 banks) + 2 y banks ----
    ps_sp = [nc.alloc_psum_tensor(f"ps_s{i}", [C, 2, CH], F32).ap()
             for i in range(3)]
    ps_yp = [nc.alloc_psum_tensor(f"ps_y{i}", [C, CH], F32).ap()
             for i in range(2)]

    def s_bank(bc):
        return ps_sp[(bc // 2) % 3][:, bc % 2, :]

    # ---- semaphores ----
    sem_names = (["sdx0", "sdx1", "sdw", "sp", "sa", "sv", "sg",
                  "syd0", "syd1"]
                 + [f"sgw{j}" for j in range(4)]
                 + [f"swt{j}" for j in range(4)]
                 + [f"sax{j}" for j in range(4)])
    sems = {n: nc.alloc_semaphore(n) for n in sem_names}

    ENGS = ("sync", "pe", "act", "dve", "gp")
    plan = {e: [] for e in ENGS}
    cnt = {n: 0 for n in sem_names}

    def op(eng, fn, sem, inc=1):
        plan[eng].append(("op", fn, sem, inc))
        if sem:
            cnt[sem] += inc
            return (sem, cnt[sem])
        return None

    def wait(eng, mark):
        if mark:
            s_, v = mark
            if v > 0:
                plan[eng].append(("w", s_, v))

    RELU = mybir.ActivationFunctionType.Relu
    EXP = mybir.ActivationFunctionType.Exp
    ADD = mybir.AluOpType.add
    MAX = mybir.AluOpType.max

    # ================= input DMAs =================
    XP0 = op("sync", lambda: nc.sync.dma_start(
        out=xs[:, 0:1536], in_=xs_d[:, 0:1536]), "sdx0", 16)
    XP1 = op("sync", lambda: nc.sync.dma_start(
        out=xs[:, 1536:RECTA], in_=xs_d[:, 1536:RECTA]), "sdx1", 16)
    for dst, srcd in ((wall, wall_d), (lr, lr_d), (sm, sm_d), (bias, bias_d)):
        SDW = op("sync", lambda d=dst, s=srcd:
                 nc.sync.dma_start(out=d, in_=s), "sdw", 16)

    ESHM = op("dve", lambda: nc.vector.memset(esh, ESH), "sv")

    # one-time: ae oob column + aa tail zeros (gpsimd, SBUF only)
    oob_bc = bass.AP(tensor=bias.tensor, offset=oobc.offset,
                     ap=[[6, C], [0, NB], [1, 1]])
    wait("gp", SDW)
    AEOOB = op("gp", lambda: nc.gpsimd.tensor_copy(
        ae[:, :, NWR:NWR + 1], oob_bc), "sg")
    AATAIL = op("gp", lambda: nc.gpsimd.memset(aa[:, :, NWR:NW], 0.0), "sg")

    # ================= convs (pair granularity) =================
    xsr = xs[:, 0:RECT].rearrange("p (c r) -> p c r", r=HR)

    def rect_rhs(src):
        return lambda j: src[:, bass.ts(j, CH)]

    def q1_rhs(j):
        return xsr[:, PAD + 32 * j:PAD + 32 * (j + 1), PAD:PAD + RPC]

    conv_list = {
        "k1": ("wk1t", rect_rhs(xs), k1, "bk1"),
        "q1": ("wq1t", q1_rhs, q1, "bq1"),
        "k2": ("wk2t", rect_rhs(k1), kpad, "bk2"),
        "v": ("wvt", rect_rhs(xs), vpad, "bv"),
        "q2": ("wq2t", rect_rhs(q1), q, "bq2"),
        "wav": ("wavw", rect_rhs(vpad), wav, None),
    }
    src_of = {"k2": "k1", "q2": "q1", "wav": "v"}

    epi_done = {}        # (cname, pair) -> mark
    last_pt_user = {}    # pair-tensor idx -> mark
    pt_i = [0]
    mems_k, mems_v = [], []

    # epilogue engine per (conv, pair): ACT except two DVE split-op pairs
    DVE_EPIS = {("k1", 1), ("v", 1)}

    def emit_conv_pair(cname, s):
        wn, rhsf, dst, bn = conv_list[cname]
        pt = pt_i[0] % 3
        pt_i[0] += 1
        ps2 = ps_sp[pt]
        if cname in src_of:
            wait("pe", epi_done.get((src_of[cname], s)))
        if cname == "wav":
            for mk_ in mems_v:
                wait("pe", mk_)
        wait("pe", last_pt_user.get(pt))
        if cname == "k1" or cname == "q1":
            if (cname, s) in (("k1", 0), ("q1", 0)):
                wait("pe", XP0)
                wait("pe", SDW)
            else:
                wait("pe", XP1)
        mm = None
        for h in (0, 1):
            j = 2 * s + h
            mm = op("pe", lambda p=ps2[:, h, :], w_=w_sb[wn], r=rhsf(j):
                    nc.tensor.matmul(p, w_, r, start=True, stop=True), "sp")
        dpair = dst[:, 1024 * s:1024 * (s + 1)].rearrange(
            "p (a b) -> p a b", b=CH)
        if bn is None:
            wait("dve", mm)
            mk = op("dve", lambda o=dpair, p=ps2:
                    nc.vector.tensor_copy(o, p), "sv")
        elif (cname, s) in DVE_EPIS:
            wait("dve", mm)
            cpm = op("dve", lambda o=escr, p=ps2:
                     nc.vector.tensor_copy(
                         o.rearrange("p (a b) -> p a b", b=CH), p), "sv")
            wait("dve", cpm)
            mk = op("dve", lambda o=dst[:, 1024 * s:1024 * (s + 1)], i_=escr,
                    b=b_col[bn]:
                    nc.vector.tensor_scalar(o, i_, b, 0.0, ADD, MAX), "sv")
        else:
            wait("act", mm)
            mk = op("act", lambda o=dpair, p=ps2, b=b_col[bn]:
                    nc.scalar.activation(o, p, RELU, bias=b), "sa")
        epi_done[(cname, s)] = mk
        last_pt_user[pt] = mk

    for s in range(3):
        emit_conv_pair("k1", s)
        if s < 2:
            emit_conv_pair("q1", s)
    for s in range(3):
        emit_conv_pair("k2", s)
        emit_conv_pair("v", s)

    # pad-col memsets after the epilogues that wrote garbage there
    kpr = kpad[:, 0:RECT].rearrange("p (c r) -> p c r", r=HR)
    vpr = vpad[:, 0:RECT].rearrange("p (c r) -> p c r", r=HR)
    for t, lst, nm in ((kpr, mems_k, "k2"), (vpr, mems_v, "v")):
        wait("gp", epi_done[(nm, 0)])
        lst.append(op("gp", lambda tf=t[:, 0:PAD, :]:
                      nc.gpsimd.memset(tf, 0.0), "sg"))
        wait("gp", epi_done[(nm, 2)])
        lst.append(op("gp", lambda tf=t[:, PAD + W:WPC, :]:
                      nc.gpsimd.memset(tf, 0.0), "sg"))

    for s in range(2):
        emit_conv_pair("q2", s)
    for s in range(3):
        emit_conv_pair("wav", s)

    # ===== wav gathers (gpsimd queue) + wavt XBARs (act queue) ==========
    WPAIR_HI = [0, 1, 2, 2]   # wav pair covering group g's windows
    gw_mark, wvt_mark = {}, {}
    for g in range(4):
        wait("gp", epi_done[("wav", WPAIR_HI[g])])
        if g >= 2:
            wait("gp", (f"swt{g - 2}", 16))
        base = wav[:, WSTRIDE * 4 * g:WSTRIDE * 4 * g + NW]
        win = bass.AP(tensor=wav.tensor, offset=base.offset,
                      ap=[[RECTA, C], [WSTRIDE, 4], [1, NW]])
        gw_mark[g] = op("gp", lambda o=wavg[:, g % 2], i_=win:
                        nc.gpsimd.dma_start(out=o, in_=i_), f"sgw{g}", 16)
    for g in range(4):
        wait("act", gw_mark[g])
        wvt_mark[g] = op("act", lambda o=wavt[:, 12 * g:12 * (g + 1), :],
                         i_=wavg[:, g % 2]:
                         nc.scalar.dma_start(out=o, in_=i_, transpose=True),
                         f"swt{g}", 16)

    # ================= attention =================
    s_done, exp_done, norm_done, ax_mark, grp_done = {}, {}, {}, {}, {}
    ydma, ycopy = {}, {}

    def st_s(bc):
        ps = s_bank(bc)[:, 0:NWR]
        wait("pe", epi_done[("q2", bc // 8)])
        wait("pe", epi_done[("k2", min((WSTRIDE * bc + NWR - 1) // 1024, 2))])
        for mk_ in mems_k:
            wait("pe", mk_)
        if bc >= 6:
            wait("pe", exp_done[(bc - 6) // 2])
        else:
            wait("pe", last_pt_user.get((bc // 2) % 3))
        op("pe", lambda o=ps, l=q[:, bass.ts(bc, BPX)],
                 r=kpad[:, bass.ds(WSTRIDE * bc, NWR)]:
           nc.tensor.matmul(o, l, r, start=True, stop=False), "sp")
        s_done[bc] = op("pe", lambda o=ps, l=Lm, r=Rm:
                        nc.tensor.matmul(o, l, r, start=False, stop=True),
                        "sp")

    def st_exp(i):  # pair i: blocks 2i, 2i+1
        bc = 2 * i
        wait("act", s_done[bc + 1])
        wait("act", ESHM)
        exp_done[i] = op(
            "act",
            lambda o=ae[:, bc:bc + 2, 0:NWR], i_=ps_sp[i % 3][:, :, 0:NWR]:
                nc.scalar.activation(o, i_, EXP, bias=esh), "sa")

    def st_softmax_group(g):  # blocks 4g..4g+3
        c0 = 4 * (g % 2)
        wait("dve", exp_done[2 * g + 1])
        wait("dve", AEOOB)
        zr = op("dve", lambda o=zz[:, c0:c0 + 4],
                i_=ae[:, 4 * g:4 * g + 4, 0:NWR + 1]:
                nc.vector.reduce_sum(o, i_, axis=mybir.AxisListType.X), "sv")
        wait("dve", zr)
        rc = op("dve", lambda o=rz[:, c0:c0 + 4], i_=zz[:, c0:c0 + 4]:
                nc.vector.reciprocal(o, i_), "sv")
        wait("dve", rc)
        rzb = bass.AP(tensor=rz.tensor, offset=rz[:, c0:c0 + 4].offset,
                      ap=[[8, C], [1, 4], [0, NWR]])
        norm_done[g] = op(
            "dve",
            lambda o=aa[:, 4 * g:4 * g + 4, 0:NWR],
                   i_=ae[:, 4 * g:4 * g + 4, 0:NWR], r=rzb:
                nc.vector.tensor_mul(o, i_, r), "sv")

    def st_ax(g):
        wait("sync", norm_done[g])
        wait("sync", AATAIL)
        ax_mark[g] = op(
            "sync",
            lambda o=at[:, 12 * g:12 * (g + 1), :], i_=aa[:, 4 * g:4 * g + 4, :]:
                nc.sync.dma_start(out=o, in_=i_, transpose=True),
            f"sax{g}", 16)

    def st_group(g):
        pq = g % 2
        ps = ps_yp[pq]
        if g >= 2:
            wait("pe", ycopy[g - 2])
        else:
            wait("pe", last_pt_user.get(None))  # no-op
        op("pe", lambda o=ps, l=w_sb["wxt"],
                 r=xsr[:, PAD + 32 * g:PAD + 32 * (g + 1), PAD:PAD + RPC]:
           nc.tensor.matmul(o, l, r, start=True, stop=False,
                            skip_group_check=True), "sp")
        op("pe", lambda o=ps, l=bo_row, r=ones_row:
           nc.tensor.matmul(o, l, r, start=False, stop=False,
                            skip_group_check=True), "sp")
        wait("pe", (f"sax{g}", 16))
        wait("pe", (f"swt{g}", 16))
        last = None
        for i in range(4):
            bc = 4 * g + i
            for t in range(3):
                fin = (i == 3 and t == 2)
                last = op(
                    "pe",
                    lambda o=ps[:, bass.ts(i, BPX)],
                           l=wavt[:, 3 * bc + t, :], r=at[:, 3 * bc + t, :],
                           sp_=fin:
                        nc.tensor.matmul(o, l, r, start=False, stop=sp_,
                                         skip_group_check=True), "sp")
        grp_done[g] = last

    for bc in range(NB):
        st_s(bc)
        if bc % 2 == 1:
            st_exp(bc // 2)
        if bc % 4 == 3:
            st_softmax_group(bc // 4)
            st_ax(bc // 4)

    for g in range(4):
        st_group(g)
        ceng = ("act", "dve")[g % 2]
        wait(ceng, grp_done[g])
        if g >= 2:
            wait(ceng, ydma[g - 2])
        if ceng == "act":
            ycopy[g] = op("act", lambda o=yt[:, g % 2, :], i_=ps_yp[g % 2]:
                          nc.scalar.copy(o, i_), "sa")
        else:
            ycopy[g] = op("dve", lambda o=yt[:, g % 2, :], i_=ps_yp[g % 2]:
                          nc.vector.tensor_copy(o, i_), "sv")
        wait("sync", ycopy[g])
        ydma[g] = op(
            "sync",
            lambda o=y_d[:, bass.ts(g, CH)], i_=yt[:, g % 2, :]:
                nc.sync.dma_start(out=o, in_=i_),
            f"syd{g % 2}", 16)

    # ---- tail barrier ----
    for s_ in ("sp", "sa", "sv", "sg"):
        wait("sync", (s_, cnt[s_]))
    for s_ in ("sdx0", "sdx1", "sdw", "syd0", "syd1"):
        wait("sync", (s_, cnt[s_]))
    for j in range(4):
        for p_ in ("sgw", "swt", "sax"):
            wait("sync", (f"{p_}{j}", cnt[f"{p_}{j}"]))

    # ---- emit ----
    def run(eng_name, eng_obj):
        hwm = {}
        for item in plan[eng_name]:
            if item[0] == "w":
                _, s_, v = item
                if hwm.get(s_, 0) >= v:
                    continue
                hwm[s_] = v
                eng_obj.wait_ge(sems[s_], v)
            else:
                _, fn, s_, inc = item
                inst = fn()
                if s_:
                    inst.then_inc(sems[s_], inc)

    with nc.Block() as block:
        @block.sync
        def _(e):
            run("sync", e)

        @block.tensor
        def _(e):
            run("pe", e)

        @block.scalar
        def _(e):
            run("act", e)

        @block.vector
        def _(e):
            run("dve", e)

        @block.gpsimd
        def _(e):
            run("gp", e)

    with nc.Block() as block2:
        @block2.sync
        def _(e):
            for n in sem_names:
                nc.sync.sem_clear(sems[n])

    return nc


_PROGRAM = None


def _host_inputs(x, w_q1, s_q1, b_q1, w_q2, s_q2, b_q2,
                 w_k1, s_k1, b_k1, w_k2, s_k2, b_k2,
                 w_v, s_v, b_v, w_o, s_o, b_o):
    def foldT(w, s):
        return np.ascontiguousarray((s[:, None] * w).T.astype(ml_dtypes.bfloat16))

    wq1t, wq2t = foldT(w_q1, s_q1), foldT(w_q2, s_q2)
    wk1t, wk2t = foldT(w_k1, s_k1), foldT(w_k2, s_k2)
    wvt = foldT(w_v, s_v)
    wo = s_o[:, None] * np.asarray(w_o, np.float32)
    wat = np.ascontiguousarray(wo[:, :C].T.astype(ml_dtypes.bfloat16))
    wxt = np.ascontiguousarray(wo[:, C:].T.astype(ml_dtypes.bfloat16))
    wall = np.concatenate([wk1t, wq1t, wk2t, wvt, wq2t, wxt, wat], axis=1)

    L = np.zeros((MR, BPX), np.float32)
    for p in range(BPX):
        pc, pr = p // RPC, p % RPC
        L[pc, p] = 1.0
        L[8 + pr, p] = 1.0
    e16 = np.float32(np.exp(ESH))

    sm = np.zeros((1, C + CH), np.float32)
    sm[0, :C] = b_o
    sm[0, C:] = 1.0

    X = np.asarray(x, np.float32).reshape(C, H, W)
    shared = dict(
        wall=np.ascontiguousarray(wall),
        sm=np.ascontiguousarray(sm.astype(ml_dtypes.bfloat16)),
    )

    col = lambda b: b.astype(np.float32)[:, None]
    in_maps = []
    for core in range(NCORES):
        h0 = core * RPC
        rect = np.zeros((C, HR, WPC), np.float32)
        lo, hi = h0 - PAD, h0 + RPC + PAD
        slo, shi = max(lo, 0), min(hi, H)
        rect[:, slo - lo:shi - lo, PAD:PAD + W] = X[:, slo:shi, :]
        xs_cm = np.zeros((C, RECTA), np.float32)
        xs_cm[:, :RECT] = rect.transpose(0, 2, 1).reshape(C, RECT)

        rowok = np.array([0 <= h0 + nr - PAD < H for nr in range(HR)])
        R = np.zeros((MR, NWR), np.float32)
        for n in range(NWR):
            ncol, nrow = n // HR, n % HR
            for k in range(8):
                if not (k <= ncol <= k + 6):
                    R[k, n] = MBIG
            for j in range(RPC):
                if not (j <= nrow <= j + 6 and rowok[nrow]):
                    R[8 + j, n] = MBIG
        lrm = np.concatenate([L, R], axis=1)

        oob = np.zeros((C, 1), np.float32)
        for p in range(BPX):
            pr = p % RPC
            n_oob = sum(1 for i in range(7) if not (0 <= h0 + pr - PAD + i < H))
            oob[p, 0] = 7 * n_oob * e16

        biases = np.concatenate(
            [col(b_k1), col(b_q1), col(b_k2), col(b_v), col(b_q2), oob], axis=1)

        m = dict(shared)
        m["xs"] = np.ascontiguousarray(xs_cm.astype(ml_dtypes.bfloat16))
        m["lr"] = np.ascontiguousarray(lrm.astype(ml_dtypes.bfloat16))
        m["bias"] = np.ascontiguousarray(biases.astype(np.float32))
        in_maps.append(m)
    return in_maps


def kernel(**inputs):
    global _PROGRAM
    if _PROGRAM is None:
        _PROGRAM = _build_program()
    in_maps = _host_inputs(**{k: np.asarray(v) for k, v in inputs.items()})
    res = run_bass_kernel_spmd(_PROGRAM, in_maps, core_ids=list(range(NCORES)))
    stripes = []
    for r in res.results:
        y = np.asarray(r["y"]).reshape(C, W, RPC)
        stripes.append(y.transpose(0, 2, 1))
    return np.concatenate(stripes, axis=1).reshape(1, C, H, W)


if __name__ == "__main__":
    rng = np.random.default_rng(0)
    fake = {"x": rng.standard_normal((1, C, H, W), np.float32)}
    for n in ("q1", "q2", "k1", "k2", "v", "o"):
        cin = 2 * C if n == "o" else C
        fake["w_" + n] = (rng.standard_normal((C, cin)) / np.sqrt(cin)).astype(np.float32)
        fake["s_" + n] = rng.uniform(0.5, 1.5, C).astype(np.float32)
        fake["b_" + n] = (rng.standard_normal(C) * 0.1).astype(np.float32)
    out = kernel(**fake)
    print("kernel output", out.shape, out.dtype)


# revision 5
# speedup vs baseline: 1.5796x; 1.1165x over previous
"""Trainium2 Bass kernel for ModalitySpecificLocalSelfAttention (7x7 window).

v3: col-major layout (see v2 docstring) with per-instruction overheads
minimized after hardware profiling:
  - conv epilogues run on PAIRS of PSUM banks ([C,1024] per op), ACT mostly;
  - softmax stats at GROUP granularity (4 blocks): one reduce (Z includes a
    pre-written oob column), one reciprocal, one stride-0-broadcast
    tensor_mul normalize;
  - all 32 per-block XBAR transposes collapsed to 4 strided gather DMAs +
    4 batched wavt XBARs (gpsimd-issued) and 4 batched a-XBARs (sync);
  - S/mask matmuls at width 308 (tails handled by one-time memsets).
"""

import sys

for _p in ("/opt/trn_rl_repo", "/root/.axon_site/_ro/trn_rl_repo"):
    if _p not in sys.path:
        sys.path.append(_p)

import ml_dtypes
import numpy as np

import concourse.bass as bass
from concourse import mybir
from concourse.bass_utils import run_bass_kernel_spmd

F32 = mybir.dt.float32
BF16 = mybir.dt.bfloat16

C = 128
H = 128
W = 128
NCORES = 8
RPC = H // NCORES          # 16 rows per core
PAD = 3
HR = RPC + 2 * PAD         # 22 halo rows
WPC = W + 2 * PAD          # 134 padded cols
RECT = HR * WPC            # 2948
RECTA = 3072               # allocated (24 x 128)
OWN = RPC * W              # 2048 owned pixels
NB = 16                    # blocks of 8 image cols x 16 rows
BPX = 128                  # pixels per block
NW = 384                   # padded neighborhood width (real 308)
NWR = 308                  # real neighborhood (14 cols x 22 rows)
WSTRIDE = 8 * HR           # 176
MR = 24                    # mask rank
CH = 512                   # conv chunk (one PSUM bank of f32)
ESH = -16.0
MBIG = -100.0

NKC = RECTA // CH          # 6 chunks (3 pairs) rect convs
NQC = OWN // CH            # 4 chunks (2 pairs) owned convs


def _build_program():
    nc = bass.Bass("TRN2", target_bir_lowering=False, debug=False)

    # ---- DRAM I/O ----
    xs_d = nc.dram_tensor("xs", [C, RECTA], BF16, kind="ExternalInput").ap()
    wall_d = nc.dram_tensor("wall", [C, 7 * C], BF16, kind="ExternalInput").ap()
    lr_d = nc.dram_tensor("lr", [MR, BPX + NWR], BF16, kind="ExternalInput").ap()
    sm_d = nc.dram_tensor("sm", [1, C + CH], BF16, kind="ExternalInput").ap()
    bias_d = nc.dram_tensor("bias", [C, 7], F32, kind="ExternalInput").ap()
    y_d = nc.dram_tensor("y", [C, OWN], F32, kind="ExternalOutput").ap()

    # ---- SBUF ----
    sb = lambda name, shape, dt: nc.alloc_sbuf_tensor(name, list(shape), dt).ap()
    xs = sb("xs_sb", [C, RECTA], BF16)
    k1 = sb("k1_sb", [C, RECTA], BF16)
    kpad = sb("kpad_sb", [C, RECTA], BF16)
    vpad = sb("vpad_sb", [C, RECTA], BF16)
    wav = sb("wav_sb", [C, RECTA], BF16)
    q1 = sb("q1_sb", [C, OWN], BF16)
    q = sb("q_sb", [C, OWN], BF16)
    wall = sb("wall_sb", [C, 7 * C], BF16)
    lr = sb("lr_sb", [MR, BPX + NWR], BF16)
    sm = sb("sm_sb", [1, C + CH], BF16)
    bias = sb("bias_sb", [C, 7], F32)
    ae = sb("ae_sb", [C, NB, NW], BF16)      # exp; col 308 = oob Z term
    aa = sb("aa_sb", [C, NB, NW], BF16)      # normalized attention
    at = sb("at_sb", [C, 3 * NB, C], BF16)
    wavt = sb("wavt_sb", [C, 3 * NB, C], BF16)
    wavg = sb("wavg_sb", [C, 2, 4, NW], BF16)  # gathered wav windows (parity)
    zz = sb("zz_sb", [C, 8], F32)
    rz = sb("rz_sb", [C, 8], F32)
    yt = sb("yt_sb", [C, 2, CH], F32)
    esh = sb("esh_sb", [C, 1], F32)
    escr = sb("escr_sb", [C, 2 * CH], BF16)

    W_NAMES = ("wk1t", "wq1t", "wk2t", "wvt", "wq2t", "wxt", "wavw")
    w_sb = {n: wall[:, bass.ts(i, C)] for i, n in enumerate(W_NAMES)}
    Lm = lr[:, 0:BPX]
    Rm = lr[:, BPX:BPX + NWR]
    bo_row = sm[:, 0:C]
    ones_row = sm[:, C:C + CH]
    b_col = {n: bias[:, i:i + 1]
             for i, n in enumerate(("bk1", "bq1", "bk2", "bv", "bq2"))}
    oobc = bias[:, 5:6]
    bo_col = bias[:, 6:7]

    # ---- PSUM: 3 conv/S pair-tensors (6 banks) + 2 y banks ----
    ps_sp = [nc.alloc_psum_tensor(f"ps_s{i}", [C, 2, CH], F32).ap()
             for i in range(3)]
    ps_yp = [nc.alloc_psum_tensor(f"ps_y{i}", [C, CH], F32).ap()
             for i in range(2)]

    def s_bank(bc):
        return ps_sp[(bc // 2) % 3][:, bc % 2, :]

    # ---- semaphores ----
    sem_names = (["sdx0", "sdx1", "sdw", "sp", "sa", "sv", "sg",
                  "syd0", "syd1"]
                 + [f"sgw{j}" for j in range(4)]
                 + [f"swt{j}" for j in range(4)]
                 + [f"sax{j}" for j in range(4)])
    sems = {n: nc.alloc_semaphore(n) for n in sem_names}

    ENGS = ("sync", "pe", "act", "dve", "gp")
    plan = {e: [] for e in ENGS}
    cnt = {n: 0 for n in sem_names}

    def op(eng, fn, sem, inc=1):
        plan[eng].append(("op", fn, sem, inc))
        if sem:
            cnt[sem] += inc
            return (sem, cnt[sem])
        return None

    def wait(eng, mark):
        if mark:
            s_, v = mark
            if v > 0:
                plan[eng].append(("w", s_, v))

    RELU = mybir.ActivationFunctionType.Relu
    EXP = mybir.ActivationFunctionType.Exp
    ADD = mybir.AluOpType.add
    MAX = mybir.AluOpType.max

    # ================= input DMAs =================
    XP0 = op("sync", lambda: nc.sync.dma_start(
        out=xs[:, 0:1536], in_=xs_d[:, 0:1536]), "sdx0", 16)
    XP1 = op("sync", lambda: nc.sync.dma_start(
        out=xs[:, 1536:RECTA], in_=xs_d[:, 1536:RECTA]), "sdx1", 16)
    for dst, srcd in ((wall, wall_d), (lr, lr_d), (sm, sm_d), (bias, bias_d)):
        SDW = op("sync", lambda d=dst, s=srcd:
                 nc.sync.dma_start(out=d, in_=s), "sdw", 16)

    ESHM = op("dve", lambda: nc.vector.memset(esh, ESH), "sv")

    # one-time: ae oob column + aa tail zeros (gpsimd, SBUF only)
    oob_bc = bass.AP(tensor=bias.tensor, offset=oobc.offset,
                     ap=[[7, C], [0, NB], [1, 1]])
    wait("gp", SDW)
    AEOOB = op("gp", lambda: nc.gpsimd.tensor_copy(
        ae[:, :, NWR:NWR + 1], oob_bc), "sg")
    AATAIL = op("gp", lambda: nc.gpsimd.memset(aa[:, :, NWR:NW], 0.0), "sg")

    # ================= convs (pair granularity) =================
    xsr = xs[:, 0:RECT].rearrange("p (c r) -> p c r", r=HR)

    def rect_rhs(src):
        return lambda j: src[:, bass.ts(j, CH)]

    def q1_rhs(j):
        return xsr[:, PAD + 32 * j:PAD + 32 * (j + 1), PAD:PAD + RPC]

    conv_list = {
        "k1": ("wk1t", rect_rhs(xs), k1, "bk1"),
        "q1": ("wq1t", q1_rhs, q1, "bq1"),
        "k2": ("wk2t", rect_rhs(k1), kpad, "bk2"),
        "v": ("wvt", rect_rhs(xs), vpad, "bv"),
        "q2": ("wq2t", rect_rhs(q1), q, "bq2"),
        "wav": ("wavw", rect_rhs(vpad), wav, None),
    }
    src_of = {"k2": "k1", "q2": "q1", "wav": "v"}

    epi_done = {}        # (cname, pair) -> mark
    last_pt_user = {}    # pair-tensor idx -> mark
    pt_i = [0]
    mems_k, mems_v = [], []

    # epilogue engine per (conv, pair): ACT except two DVE split-op pairs
    DVE_EPIS = {("k1", 1), ("v", 1)}

    def emit_conv_pair(cname, s):
        wn, rhsf, dst, bn = conv_list[cname]
        pt = pt_i[0] % 3
        pt_i[0] += 1
        ps2 = ps_sp[pt]
        if cname in src_of:
            wait("pe", epi_done.get((src_of[cname], s)))
        if cname == "wav":
            for mk_ in mems_v:
                wait("pe", mk_)
        wait("pe", last_pt_user.get(pt))
        if cname == "k1" or cname == "q1":
            if (cname, s) in (("k1", 0), ("q1", 0)):
                wait("pe", XP0)
                wait("pe", SDW)
            else:
                wait("pe", XP1)
        mm = None
        for h in (0, 1):
            j = 2 * s + h
            mm = op("pe", lambda p=ps2[:, h, :], w_=w_sb[wn], r=rhsf(j):
                    nc.tensor.matmul(p, w_, r, start=True, stop=True), "sp")
        dpair = dst[:, 1024 * s:1024 * (s + 1)].rearrange(
            "p (a b) -> p a b", b=CH)
        if bn is None:
            wait("dve", mm)
            mk = op("dve", lambda o=dpair, p=ps2:
                    nc.vector.tensor_copy(o, p), "sv")
        elif (cname, s) in DVE_EPIS:
            wait("dve", mm)
            cpm = op("dve", lambda o=escr, p=ps2:
                     nc.vector.tensor_copy(
                         o.rearrange("p (a b) -> p a b", b=CH), p), "sv")
            wait("dve", cpm)
            mk = op("dve", lambda o=dst[:, 1024 * s:1024 * (s + 1)], i_=escr,
                    b=b_col[bn]:
                    nc.vector.tensor_scalar(o, i_, b, 0.0, ADD, MAX), "sv")
        else:
            wait("act", mm)
            mk = op("act", lambda o=dpair, p=ps2, b=b_col[bn]:
                    nc.scalar.activation(o, p, RELU, bias=b), "sa")
        epi_done[(cname, s)] = mk
        last_pt_user[pt] = mk

    for s in range(3):
        emit_conv_pair("k1", s)
        if s < 2:
            emit_conv_pair("q1", s)
    for s in range(3):
        emit_conv_pair("k2", s)
        emit_conv_pair("v", s)

    # pad-col memsets after the epilogues that wrote garbage there
    kpr = kpad[:, 0:RECT].rearrange("p (c r) -> p c r", r=HR)
    vpr = vpad[:, 0:RECT].rearrange("p (c r) -> p c r", r=HR)
    for t, lst, nm in ((kpr, mems_k, "k2"), (vpr, mems_v, "v")):
        wait("gp", epi_done[(nm, 0)])
        lst.append(op("gp", lambda tf=t[:, 0:PAD, :]:
                      nc.gpsimd.memset(tf, 0.0), "sg"))
        wait("gp", epi_done[(nm, 2)])
        lst.append(op("gp", lambda tf=t[:, PAD + W:WPC, :]:
                      nc.gpsimd.memset(tf, 0.0), "sg"))

    for s in range(2):
        emit_conv_pair("q2", s)
    for s in range(3):
        emit_conv_pair("wav", s)

    # ===== wav gathers (gpsimd queue) + wavt XBARs (act queue) ==========
    WPAIR_HI = [0, 1, 2, 2]   # wav pair covering group g's windows
    gw_mark, wvt_mark = {}, {}
    for g in range(4):
        wait("gp", epi_done[("wav", WPAIR_HI[g])])
        if g >= 2:
            wait("gp", (f"swt{g - 2}", 16))
        base = wav[:, WSTRIDE * 4 * g:WSTRIDE * 4 * g + NW]
        win = bass.AP(tensor=wav.tensor, offset=base.offset,
                      ap=[[RECTA, C], [WSTRIDE, 4], [1, NW]])
        gw_mark[g] = op("gp", lambda o=wavg[:, g % 2], i_=win:
                        nc.gpsimd.dma_start(out=o, in_=i_), f"sgw{g}", 16)
    for g in range(4):
        wait("sync", gw_mark[g])
        wvt_mark[g] = op("sync", lambda o=wavt[:, 12 * g:12 * (g + 1), :],
                         i_=wavg[:, g % 2]:
                         nc.sync.dma_start(out=o, in_=i_, transpose=True),
                         f"swt{g}", 16)

    # ================= attention =================
    s_done, exp_done, norm_done, ax_mark, grp_done = {}, {}, {}, {}, {}
    ydma, ycopy = {}, {}

    def st_s(bc):
        ps = s_bank(bc)[:, 0:NWR]
        if bc == 0:
            wait("pe", epi_done[("q2", 1)])
            wait("pe", epi_done[("k2", 2)])
            for mk_ in mems_k:
                wait("pe", mk_)
            for pt_ in range(3):
                wait("pe", last_pt_user.get(pt_))
        if bc >= 6:
            wait("pe", exp_done[(bc - 6) // 2])
        op("pe", lambda o=ps, l=q[:, bass.ts(bc, BPX)],
                 r=kpad[:, bass.ds(WSTRIDE * bc, NWR)]:
           nc.tensor.matmul(o, l, r, start=True, stop=False), "sp")
        s_done[bc] = op("pe", lambda o=ps, l=Lm, r=Rm:
                        nc.tensor.matmul(o, l, r, start=False, stop=True),
                        "sp")

    def st_exp(i):  # pair i: blocks 2i, 2i+1
        bc = 2 * i
        wait("act", s_done[bc + 1])
        wait("act", ESHM)
        exp_done[i] = op(
            "act",
            lambda o=ae[:, bc:bc + 2, 0:NWR], i_=ps_sp[i % 3][:, :, 0:NWR]:
                nc.scalar.activation(o, i_, EXP, bias=esh), "sa")

    def st_softmax_group(g):  # blocks 4g..4g+3
        c0 = 4 * (g % 2)
        wait("dve", exp_done[2 * g + 1])
        wait("dve", AEOOB)
        zr = op("dve", lambda o=zz[:, c0:c0 + 4],
                i_=ae[:, 4 * g:4 * g + 4, 0:NWR + 1]:
                nc.vector.reduce_sum(o, i_, axis=mybir.AxisListType.X), "sv")
        wait("dve", zr)
        rc = op("dve", lambda o=rz[:, c0:c0 + 4], i_=zz[:, c0:c0 + 4]:
                nc.vector.reciprocal(o, i_), "sv")
        wait("dve", rc)
        rzb = bass.AP(tensor=rz.tensor, offset=rz[:, c0:c0 + 4].offset,
                      ap=[[8, C], [1, 4], [0, NWR]])
        norm_done[g] = op(
            "dve",
            lambda o=aa[:, 4 * g:4 * g + 4, 0:NWR],
                   i_=ae[:, 4 * g:4 * g + 4, 0:NWR], r=rzb:
                nc.vector.tensor_mul(o, i_, r), "sv")

    def st_ax(g):
        wait("sync", norm_done[g])
        wait("sync", AATAIL)
        ax_mark[g] = op(
            "sync",
            lambda o=at[:, 12 * g:12 * (g + 1), :], i_=aa[:, 4 * g:4 * g + 4, :]:
                nc.sync.dma_start(out=o, in_=i_, transpose=True),
            f"sax{g}", 16)

    def st_group(g):
        pq = g % 2
        ps = ps_yp[pq]
        if g >= 2:
            wait("pe", ycopy[g - 2])
        op("pe", lambda o=ps, l=w_sb["wxt"],
                 r=xsr[:, PAD + 32 * g:PAD + 32 * (g + 1), PAD:PAD + RPC]:
           nc.tensor.matmul(o, l, r, start=True, stop=False,
                            skip_group_check=True), "sp")
        wait("pe", (f"sax{g}", 16))
        wait("pe", (f"swt{g}", 16))
        last = None
        for i in range(4):
            bc = 4 * g + i
            for t in range(3):
                fin = (i == 3 and t == 2)
                last = op(
                    "pe",
                    lambda o=ps[:, bass.ts(i, BPX)],
                           l=wavt[:, 3 * bc + t, :], r=at[:, 3 * bc + t, :],
                           sp_=fin:
                        nc.tensor.matmul(o, l, r, start=False, stop=sp_,
                                         skip_group_check=True), "sp")
        grp_done[g] = last

    for bc in range(NB):
        st_s(bc)
        if bc % 2 == 1:
            st_exp(bc // 2)
        if bc % 4 == 3:
            st_softmax_group(bc // 4)
            st_ax(bc // 4)

    for g in range(4):
        st_group(g)
        ceng = ("act", "dve")[g % 2]
        wait(ceng, grp_done[g])
        if g >= 2:
            wait(ceng, ydma[g - 2])
        if ceng == "act":
            ycopy[g] = op("act", lambda o=yt[:, g % 2, :], i_=ps_yp[g % 2]:
                          nc.scalar.activation(
                              o, i_, mybir.ActivationFunctionType.Identity,
                              bias=bo_col), "sa")
        else:
            ycopy[g] = op("dve", lambda o=yt[:, g % 2, :], i_=ps_yp[g % 2]:
                          nc.vector.tensor_scalar_add(o, i_, bo_col), "sv")
        wait("sync", ycopy[g])
        ydma[g] = op(
            "sync",
            lambda o=y_d[:, bass.ts(g, CH)], i_=yt[:, g % 2, :]:
                nc.sync.dma_start(out=o, in_=i_),
            f"syd{g % 2}", 16)

    # ---- tail barrier ----
    for s_ in ("sp", "sa", "sv", "sg"):
        wait("sync", (s_, cnt[s_]))
    for s_ in ("sdx0", "sdx1", "sdw", "syd0", "syd1"):
        wait("sync", (s_, cnt[s_]))
    for j in range(4):
        for p_ in ("sgw", "swt", "sax"):
            wait("sync", (f"{p_}{j}", cnt[f"{p_}{j}"]))

    # ---- emit ----
    def run(eng_name, eng_obj):
        hwm = {}
        for item in plan[eng_name]:
            if item[0] == "w":
                _, s_, v = item
                if hwm.get(s_, 0) >= v:
                    continue
                hwm[s_] = v
                eng_obj.wait_ge(sems[s_], v)
            else:
                _, fn, s_, inc = item
                inst = fn()
                if s_:
                    inst.then_inc(sems[s_], inc)

    with nc.Block() as block:
        @block.sync
        def _(e):
            run("sync", e)

        @block.tensor
        def _(e):
            run("pe", e)

        @block.scalar
        def _(e):
            run("act", e)

        @block.vector
        def _(e):
            run("dve", e)

        @block.gpsimd
        def _(e):
            run("gp", e)

    with nc.Block() as block2:
        @block2.sync
        def _(e):
            for n in sem_names:
                nc.sync.sem_clear(sems[n])

    return nc


_PROGRAM = None


def _host_inputs(x, w_q1, s_q1, b_q1, w_q2, s_q2, b_q2,
                 w_k1, s_k1, b_k1, w_k2, s_k2, b_k2,
                 w_v, s_v, b_v, w_o, s_o, b_o):
    def foldT(w, s):
        return np.ascontiguousarray((s[:, None] * w).T.astype(ml_dtypes.bfloat16))

    wq1t, wq2t = foldT(w_q1, s_q1), foldT(w_q2, s_q2)
    wk1t, wk2t = foldT(w_k1, s_k1), foldT(w_k2, s_k2)
    wvt = foldT(w_v, s_v)
    wo = s_o[:, None] * np.asarray(w_o, np.float32)
    wat = np.ascontiguousarray(wo[:, :C].T.astype(ml_dtypes.bfloat16))
    wxt = np.ascontiguousarray(wo[:, C:].T.astype(ml_dtypes.bfloat16))
    wall = np.concatenate([wk1t, wq1t, wk2t, wvt, wq2t, wxt, wat], axis=1)

    L = np.zeros((MR, BPX), np.float32)
    for p in range(BPX):
        pc, pr = p // RPC, p % RPC
        L[pc, p] = 1.0
        L[8 + pr, p] = 1.0
    e16 = np.float32(np.exp(ESH))

    sm = np.zeros((1, C + CH), np.float32)
    sm[0, :C] = b_o
    sm[0, C:] = 1.0

    X = np.asarray(x, np.float32).reshape(C, H, W)
    shared = dict(
        wall=np.ascontiguousarray(wall),
        sm=np.ascontiguousarray(sm.astype(ml_dtypes.bfloat16)),
    )

    col = lambda b: b.astype(np.float32)[:, None]
    in_maps = []
    for core in range(NCORES):
        h0 = core * RPC
        rect = np.zeros((C, HR, WPC), np.float32)
        lo, hi = h0 - PAD, h0 + RPC + PAD
        slo, shi = max(lo, 0), min(hi, H)
        rect[:, slo - lo:shi - lo, PAD:PAD + W] = X[:, slo:shi, :]
        xs_cm = np.zeros((C, RECTA), np.float32)
        xs_cm[:, :RECT] = rect.transpose(0, 2, 1).reshape(C, RECT)

        rowok = np.array([0 <= h0 + nr - PAD < H for nr in range(HR)])
        R = np.zeros((MR, NWR), np.float32)
        for n in range(NWR):
            ncol, nrow = n // HR, n % HR
            for k in range(8):
                if not (k <= ncol <= k + 6):
                    R[k, n] = MBIG
            for j in range(RPC):
                if not (j <= nrow <= j + 6 and rowok[nrow]):
                    R[8 + j, n] = MBIG
        lrm = np.concatenate([L, R], axis=1)

        oob = np.zeros((C, 1), np.float32)
        for p in range(BPX):
            pr = p % RPC
            n_oob = sum(1 for i in range(7) if not (0 <= h0 + pr - PAD + i < H))
            oob[p, 0] = 7 * n_oob * e16

        biases = np.concatenate(
            [col(b_k1), col(b_q1), col(b_k2), col(b_v), col(b_q2), oob,
             col(b_o)], axis=1)

        m = dict(shared)
        m["xs"] = np.ascontiguousarray(xs_cm.astype(ml_dtypes.bfloat16))
        m["lr"] = np.ascontiguousarray(lrm.astype(ml_dtypes.bfloat16))
        m["bias"] = np.ascontiguousarray(biases.astype(np.float32))
        in_maps.append(m)
    return in_maps


def kernel(**inputs):
    global _PROGRAM
    if _PROGRAM is None:
        _PROGRAM = _build_program()
    in_maps = _host_inputs(**{k: np.asarray(v) for k, v in inputs.items()})
    res = run_bass_kernel_spmd(_PROGRAM, in_maps, core_ids=list(range(NCORES)))
    stripes = []
    for r in res.results:
        y = np.asarray(r["y"]).reshape(C, W, RPC)
        stripes.append(y.transpose(0, 2, 1))
    return np.concatenate(stripes, axis=1).reshape(1, C, H, W)


if __name__ == "__main__":
    rng = np.random.default_rng(0)
    fake = {"x": rng.standard_normal((1, C, H, W), np.float32)}
    for n in ("q1", "q2", "k1", "k2", "v", "o"):
        cin = 2 * C if n == "o" else C
        fake["w_" + n] = (rng.standard_normal((C, cin)) / np.sqrt(cin)).astype(np.float32)
        fake["s_" + n] = rng.uniform(0.5, 1.5, C).astype(np.float32)
        fake["b_" + n] = (rng.standard_normal(C) * 0.1).astype(np.float32)
    out = kernel(**fake)
    print("kernel output", out.shape, out.dtype)
